# revision 1
# baseline (speedup 1.0000x reference)
"""Multi-head attention with KV-cache append, sharded over 8 trn2 NeuronCores.

Reference computation (fp32):
    qkv = x @ w_qkv + b_qkv                  # x [2,256,1024]
    q,k_new,v_new -> [B,H,N,D] (H=16, D=64)
    k_all = cat(past_k, k_new); v_all = cat(past_v, v_new)   # [B,H,8448,64]
    out = softmax(q k^T / sqrt(D)) @ v_all   # non-causal
    return out.merge_heads @ w_proj + b_proj

Sharding: tensor-parallel over heads. Core c owns heads [2c, 2c+1]:
  - past_k/past_v sharded by head, bf16 in HBM (fp8 was tried and costs
    ~3.9% rel err: the attention output is itself an incoherent average,
    so K/V quantization noise does NOT average down relative to it).
    past_k pre-transposed to [B,2,D,L]; past_v re-blocked to
    [B,128,2,L/128,D+1] (partition-major) with the softmax-denominator
    ones column baked in, so every DMA is one fully contiguous
    per-partition run.
  - w_qkv column-split / w_proj row-split per head (bf16); each core
    computes a full-shape bf16 partial of the output projection; host sums
    partials in f64 + b_proj.

Device kernel per core (all engines via the Tile framework):
  q^T/k_new^T/v_new projections on PE, then flash-style streaming over the
  KV cache. The critical loop is latency-pipelined: one [128, 1024] PSUM
  scores tile per superblock (both heads side by side) so the 3-slot PSUM
  ring holds 3 superblocks in flight, exp alternates whole tiles between
  ACT (table exp) and DVE (Schraudolph bit-trick, one tensor_scalar), and
  attn@v consumption trails scores emission by PEND superblocks -- the
  ~1.2us exp latency hides under ~2us of PE work. attn@v accumulates in
  PSUM with the ones column producing the softmax denominator row.
  Epilogue: per-head output projection of the unnormalized attention
  output, per-token denominator scaling (softmax normalization commutes
  with the projection within a head) split ACT/DVE, cross-head sum on
  gpsimd, bf16 output partials.
"""

import sys
for _p in ("/opt/trn_rl_repo", "/root/.axon_site/_ro/trn_rl_repo"):
    if _p not in sys.path:
        sys.path.append(_p)

from contextlib import ExitStack

import numpy as np

import concourse.bass as bass
import concourse.tile as tile
from concourse import bacc, mybir
from concourse.bass_utils import run_bass_kernel_spmd
from concourse.masks import make_identity

N_CORES = 8
B, N, DIM = 2, 256, 1024
H, D = 16, 64
L = 8192
HL = H // N_CORES          # 2 heads per core
DL = HL * D                # 128 local head dims
SCALE = D ** -0.5
T = B * N                  # 512 tokens
CC = DIM // 128            # 8 contraction chunks of the model dim
F32 = mybir.dt.float32
VP = D + 1                 # v padded with ones column
NBLK = 128                 # kv-cache block (l) per scores matmul
PSB = 2                    # l-blocks per PSUM superblock (per head)
NSB = L // (PSB * NBLK)    # superblocks per batch (cache part)
PEND = 3                   # attn@v trails scores by this many superblocks

MMD = mybir.dt.bfloat16
U16 = mybir.dt.uint16
F8 = mybir.dt.float8e4

# Softmax exp is ~56us of ACT work if done entirely on the scalar engine --
# the single largest serial engine cost. Offload a fraction of the exp tiles
# to the DVE using the Schraudolph bit trick in bf16:
#   bf16_bits(exp(s*SCALE)) ~= round(s * EXP_A + EXP_B)
# (one tensor_scalar mult+add, f32 PSUM in -> uint16 out, bitcast to bf16).
# Max rel err ~3.3%, rms ~1.8%, pseudo-random across keys -> averages out in
# the softmax weighted mean (common-mode cancels in the ratio entirely).
EXP_A = float(SCALE * np.log2(np.e) * 128.0)
EXP_B = 16250.5
# of every 16 exp tiles, these slot indices run on DVE (rest on ACT)
DVE_SLOTS = frozenset((1, 3, 5, 7, 9, 11, 13))

# split each attn@v matmul into two K=64 row-group-concurrent matmuls
SPLITK = False


def _emit(ctx: ExitStack, tc: tile.TileContext, aps: dict):
    nc = tc.nc
    t_x, t_k, t_v = aps["x_t"], aps["k_t"], aps["v_pad"]
    t_w, t_bq, t_bv = aps["w_loc"], aps["b_q"], aps["b_v"]
    t_wp, t_out = aps["wp_loc"], aps["out"]
    ablate = aps.get("ablate", "")

    singles = ctx.enter_context(tc.tile_pool(name="singles", bufs=1))
    kpool = ctx.enter_context(tc.tile_pool(name="kpool", bufs=2))
    vpool = ctx.enter_context(tc.tile_pool(name="vpool", bufs=2))
    apool = ctx.enter_context(tc.tile_pool(name="apool", bufs=2 * (PEND + 1)))
    epool = ctx.enter_context(tc.tile_pool(name="epool", bufs=4))
    opool = ctx.enter_context(tc.tile_pool(name="opool", bufs=6))
    rpool = ctx.enter_context(tc.tile_pool(name="rpool", bufs=8))
    scp = ctx.enter_context(tc.tile_pool(name="scp", bufs=3, space="PSUM"))
    acp = ctx.enter_context(tc.tile_pool(name="acp", bufs=2, space="PSUM"))

    Exp = mybir.ActivationFunctionType.Exp
    Ident = mybir.ActivationFunctionType.Identity
    Copy = mybir.ActivationFunctionType.Copy

    def body():
        # ---- load x^T, weights, biases ----
        # double-buffered (bufs=2) so the next For_i iteration's loads can
        # land while this iteration still reads the previous buffers
        x_sb = singles.tile([128, CC, T], MMD, tag="x", bufs=2)
        xr = t_x.rearrange("(cc p) t -> p cc t", p=128)
        nc.sync.dma_start(out=x_sb[:, 0:CC // 2], in_=xr[:, 0:CC // 2])
        nc.sync.dma_start(out=x_sb[:, CC // 2:CC], in_=xr[:, CC // 2:CC])
        # prefetch the whole KV stream now -- the transfers overlap the qkv
        # projection phase (kt on the idle SP HWDGE queue, vt on gpsimd)
        kts, vts_all = {}, {}
        for b in range(B):
            kt = kpool.tile([128, L], MMD, tag="k", name=f"kt{b}")
            nc.sync.dma_start(
                out=kt, in_=t_k[b].rearrange("h d l -> (h d) l"))
            kts[b] = kt
            vt = vpool.tile([128, HL, L // NBLK, VP], MMD, tag="v",
                            name=f"vt{b}")
            nc.gpsimd.dma_start(out=vt, in_=t_v[b])
            vts_all[b] = vt
        w_sb = singles.tile([128, CC, 3 * DL], MMD, tag="w", bufs=2)
        wr = t_w.rearrange("(cc p) d -> p cc d", p=128)
        nc.scalar.dma_start(out=w_sb[:, 0:CC // 2], in_=wr[:, 0:CC // 2])
        nc.scalar.dma_start(out=w_sb[:, CC // 2:CC], in_=wr[:, CC // 2:CC])
        wp_sb = singles.tile([D, HL, DIM], MMD, tag="wp", bufs=2)
        nc.scalar.dma_start(out=wp_sb, in_=t_wp)
        bqk_sb = singles.tile([DL, 2], F32, tag="bqk")
        nc.scalar.dma_start(out=bqk_sb, in_=t_bq)
        bq_sb, bk_sb = bqk_sb[:, 0:1], bqk_sb[:, 1:2]
        bv_sb = singles.tile([128, DL], F32, tag="bv")
        nc.gpsimd.dma_start(
            out=bv_sb,
            in_=bass.AP(tensor=t_bv.tensor, offset=0, ap=[[0, 128], [1, DL]]),
        )
        ident = singles.tile([128, 128], F32, tag="ident")
        make_identity(nc, ident)
        ones_sb = singles.tile([128, 1], F32, tag="ones")
        nc.vector.memset(ones_sb, 1.0)
        # touch Exp once while ACT is otherwise idle: pulls the ~2.7us
        # ACT_TABLE_LOAD into the startup window
        warm = singles.tile([128, 1], F32, tag="warm")
        nc.scalar.activation(warm, ones_sb, Exp)

        if ablate == "dmaonly":
            for g in range(4):
                for ech in range(2):
                    so = opool.tile([128, 512], MMD, tag="o")
                    nc.vector.tensor_copy(so, x_sb[:, g, :])
                    nc.sync.dma_start(
                        out=t_out[g * 128:(g + 1) * 128,
                                  ech * 512:(ech + 1) * 512],
                        in_=so)
            return

        at0 = None
        if ablate in ("noact", "nopev"):
            at0 = singles.tile([128, 2 * PSB * N], MMD, tag="at0")
            nc.vector.memset(at0, 0.001)
        exp_cnt = [0]

        def exp_tile(ps, cols):
            """exp of a whole-superblock scores tile (both heads)."""
            if ablate == "noact":
                return at0
            slot = exp_cnt[0] % 16
            exp_cnt[0] += 1
            if slot in DVE_SLOTS:
                atu = apool.tile([128, 2 * PSB * N], U16, tag="a", name="at")
                nc.vector.tensor_scalar(
                    atu[:, 0:cols], ps[:, 0:cols], EXP_A, EXP_B,
                    mybir.AluOpType.mult, mybir.AluOpType.add)
                return atu.bitcast(MMD)
            at = apool.tile([128, 2 * PSB * N], MMD, tag="a", name="at")
            nc.scalar.activation(at[:, 0:cols], ps[:, 0:cols], Exp,
                                 scale=SCALE)
            return at

        # ---- qkv projections ----
        # q^T, k_new^T in [d_local, token] layout (d on partitions)
        psq = scp.tile([128, T], F32, tag="sc")
        for cc in range(CC):
            nc.tensor.matmul(psq, w_sb[:, cc, 0:DL], x_sb[:, cc, :],
                             start=(cc == 0), stop=(cc == CC - 1))
        q_sb = singles.tile([DL, T], MMD, tag="q", bufs=2)
        nc.scalar.activation(q_sb, psq, Ident, bias=bq_sb)

        psk = scp.tile([128, T], F32, tag="sc")
        for cc in range(CC):
            nc.tensor.matmul(psk, w_sb[:, cc, DL:2 * DL], x_sb[:, cc, :],
                             start=(cc == 0), stop=(cc == CC - 1))
        k_sb = singles.tile([DL, T], MMD, tag="kn", bufs=2)
        nc.scalar.activation(k_sb, psk, Ident, bias=bk_sb)

        # v_new in [token, d_local] layout, padded with the ones column
        vn_sb = {}
        for nch in range(T // 128):  # 4 chunks of 128 tokens
            psv = acp.tile([128, DL], F32, tag="ac")
            for cc in range(CC):
                nc.tensor.matmul(psv, x_sb[:, cc, nch * 128:(nch + 1) * 128],
                                 w_sb[:, cc, 2 * DL:3 * DL],
                                 start=(cc == 0), stop=(cc == CC - 1))
            for h in range(HL):
                vn = singles.tile([128, VP], MMD, tag=f"vn{nch}_{h}")
                nc.vector.tensor_add(vn[:, 0:D], psv[:, h * D:(h + 1) * D],
                                     bv_sb[:, h * D:(h + 1) * D])
                nc.vector.tensor_copy(vn[:, D:VP], ones_sb)
                vn_sb[(nch, h)] = vn

        # ---- streaming attention over the KV cache ----
        for b in range(B):
            kt, vt = kts[b], vts_all[b]
            # acc [VP, 2*N]: halves accumulate the even (l in [0,64)) and
            # odd (l in [64,128)) partition-halves of each kv block via two
            # K=64 matmuls on disjoint PE row groups -- they run
            # concurrently (measured 3.07x for 4-way row tiling); the
            # epilogue sums the halves on DVE
            acc = {}
            for h in range(HL):
                acc[(b, h)] = acp.tile([VP, 2 * N], F32, tag="ac",
                                       name=f"acc{b}{h}")

            # software pipeline: attn@v of superblock i is emitted after the
            # scores of superblock i+PEND, so the exp latency (~1.2us) hides
            # under several superblocks of PE work
            pend = []

            def emit_attnv(entry):
                at, blocks = entry
                for (h, lhs_rhs) in blocks:
                    vt_ap, at_cols, start, stop = lhs_rhs
                    if SPLITK:
                        for half in range(2):
                            nc.tensor.matmul(
                                acc[(b, h)][:, half * N:(half + 1) * N],
                                vt_ap[half * D:(half + 1) * D, :],
                                at[half * D:(half + 1) * D,
                                   at_cols[0]:at_cols[1]],
                                start=start, stop=stop,
                                skip_group_check=True)
                    else:
                        nc.tensor.matmul(
                            acc[(b, h)][:, 0:N], vt_ap,
                            at[:, at_cols[0]:at_cols[1]],
                            start=start, stop=stop, skip_group_check=True)

            def flush(limit):
                while len(pend) > limit:
                    emit_attnv(pend.pop(0))

            for psb in range(NSB):
                # one [128, 2*PSB*N] PSUM tile per superblock: h0 in the
                # low half, h1 in the high half -> the 3-slot PSUM ring
                # keeps 3 superblocks in flight
                ps = scp.tile([128, 2 * PSB * N], F32, tag="sc", name="ps")
                for j in range(PSB):
                    lo = (psb * PSB + j) * NBLK
                    for h in range(HL):
                        # the two heads' scores matmuls sit on disjoint PE
                        # row groups (partitions 0-63 / 64-127) and run
                        # concurrently
                        nc.tensor.matmul(
                            ps[:, (h * PSB + j) * N:(h * PSB + j + 1) * N],
                            kt[h * D:(h + 1) * D, lo:lo + NBLK],
                            q_sb[h * D:(h + 1) * D, b * N:(b + 1) * N],
                            start=True, stop=True)
                at = exp_tile(ps, 2 * PSB * N)
                if ablate != "nopev":
                    blocks = []
                    for j in range(PSB):
                        jj = psb * PSB + j
                        for h in range(HL):
                            blocks.append((h, (
                                vt[:, h, jj, :],
                                ((h * PSB + j) * N, (h * PSB + j + 1) * N),
                                jj == 0, False)))
                    pend.append((at, blocks))
                flush(PEND)
            # new tokens (the appended k_new/v_new of this batch)
            ps = scp.tile([128, 2 * PSB * N], F32, tag="sc", name="ps")
            for j in range(2):
                for h in range(HL):
                    nc.tensor.matmul(
                        ps[:, (h * 2 + j) * N:(h * 2 + j + 1) * N],
                        k_sb[h * D:(h + 1) * D,
                             b * N + j * NBLK:b * N + (j + 1) * NBLK],
                        q_sb[h * D:(h + 1) * D, b * N:(b + 1) * N],
                        start=True, stop=True)
            at = exp_tile(ps, 4 * N)
            blocks = []
            for j in range(2):
                for h in range(HL):
                    blocks.append((h, (
                        vn_sb[(b * 2 + j, h)],
                        ((h * 2 + j) * N, (h * 2 + j + 1) * N),
                        ablate == "nopev" and j == 0, j == 1)))
            pend.append((at, blocks))
            flush(0)

            # ---- per-batch epilogue ----
            uns, recips = {}, {}
            for h in range(HL):
                # merge the even/odd row-group halves; engines may read only
                # one PSUM operand per instruction, so stage one half in SBUF
                un = epool.tile([D, N], MMD, tag="un")
                den = epool.tile([VP, N], F32, tag="den")
                if SPLITK:
                    ut = epool.tile([D, N], F32, tag="ut")
                    nc.vector.tensor_copy(ut, acc[(b, h)][0:D, 0:N])
                    nc.vector.tensor_add(un, ut, acc[(b, h)][0:D, N:2 * N])
                    nc.vector.tensor_copy(den[D:VP, :],
                                          acc[(b, h)][D:VP, 0:N])
                    nc.vector.tensor_add(den[D:VP, :], den[D:VP, :],
                                         acc[(b, h)][D:VP, N:2 * N])
                else:
                    nc.vector.tensor_copy(un, acc[(b, h)][0:D, 0:N])
                    nc.vector.tensor_copy(den[D:VP, :],
                                          acc[(b, h)][D:VP, 0:N])
                uns[h] = un
                for tch in range(N // 128):
                    tp = scp.tile([128, 1], F32, tag="sc")
                    nc.tensor.transpose(
                        tp, den[D:VP, tch * 128:(tch + 1) * 128],
                        ident[D:D + 1, D:D + 1])
                    rc = rpool.tile([128, 1], F32, tag="rc")
                    nc.vector.reciprocal(rc, tp)
                    recips[(h, tch)] = rc

            for tch in range(N // 128):
                g = b * (N // 128) + tch
                og = opool.tile([128, DIM], MMD, tag="og")
                for ech in range(DIM // 512):
                    pps = []
                    for h in range(HL):
                        pp = scp.tile([128, 512], F32, tag="sc")
                        nc.tensor.matmul(
                            pp,
                            uns[h][:, tch * 128:(tch + 1) * 128],
                            wp_sb[:, h, ech * 512:(ech + 1) * 512],
                            start=True, stop=True)
                        pps.append(pp)
                    # per-token denominator scaling, one head on ACT (Copy
                    # with a per-partition scale AP) and one on DVE
                    s0 = opool.tile([128, 512], F32, tag="o")
                    nc.scalar.activation(s0, pps[0], Copy,
                                         scale=recips[(0, tch)])
                    s1 = opool.tile([128, 512], F32, tag="o")
                    nc.vector.tensor_scalar_mul(s1, pps[1], recips[(1, tch)])
                    # cross-head sum on gpsimd (SBUF-only engine)
                    nc.gpsimd.tensor_add(og[:, ech * 512:(ech + 1) * 512],
                                         s0, s1)
                nc.sync.dma_start(
                    out=t_out[g * 128:(g + 1) * 128, :], in_=og)

    repeat = aps["repeat"]
    if repeat > 1:
        with tc.For_i(0, repeat, 1):
            body()
    else:
        body()


def build(repeat: int = 1, ablate: str = ""):
    """Build + bass-compile the SPMD program (one NeuronCore's view)."""
    nc = bacc.Bacc("TRN2", target_bir_lowering=False, debug=False,
                   num_devices=N_CORES)
    aps = {
        "x_t": nc.dram_tensor("x_t", [DIM, T], MMD, kind="ExternalInput").ap(),
        "k_t": nc.dram_tensor("k_t", [B, HL, D, L], MMD, kind="ExternalInput").ap(),
        "v_pad": nc.dram_tensor("v_pad", [B, 128, HL, L // NBLK, VP], MMD, kind="ExternalInput").ap(),
        "w_loc": nc.dram_tensor("w_loc", [DIM, 3 * DL], MMD, kind="ExternalInput").ap(),
        "b_q": nc.dram_tensor("b_qk", [DL, 2], F32, kind="ExternalInput").ap(),
        "b_v": nc.dram_tensor("b_v", [DL], F32, kind="ExternalInput").ap(),
        "wp_loc": nc.dram_tensor("wp_loc", [D, HL, DIM], MMD, kind="ExternalInput").ap(),
        "out": nc.dram_tensor("out", [T, DIM], MMD, kind="ExternalOutput").ap(),
        "repeat": repeat,
        "ablate": ablate,
    }
    with tile.TileContext(nc) as tc:
        with ExitStack() as ctx:
            _emit(ctx, tc, aps)
    nc.compile()
    return nc


def shard_inputs(x, past_k, past_v, w_qkv, b_qkv, w_proj):
    """Full inputs -> list of 8 per-core input maps (head-sharded)."""
    import ml_dtypes
    BF16 = ml_dtypes.bfloat16

    x = np.asarray(x, np.float32)
    past_k = np.asarray(past_k, np.float32)
    past_v = np.asarray(past_v, np.float32)
    w_qkv = np.asarray(w_qkv, np.float32)
    b_qkv = np.asarray(b_qkv, np.float32)
    w_proj = np.asarray(w_proj, np.float32)

    x_t = np.ascontiguousarray(x.reshape(T, DIM).T.astype(BF16))
    in_maps = []
    for c in range(N_CORES):
        lo, hi = c * DL, (c + 1) * DL
        k_t = np.ascontiguousarray(
            past_k[:, c * HL:(c + 1) * HL].transpose(0, 1, 3, 2).astype(BF16))
        # [B, HL, L, D] -> [B, HL, 128, L//128, D+1] with a ones column at
        # d=D (softmax-denominator trick): partition dim = l % 128, one
        # fully contiguous per-partition run per DMA
        # layout [B, p=128, HL, L//128, D+1]: partition-major so the SBUF
        # destination AP matches the source element order exactly
        v_blk = (past_v[:, c * HL:(c + 1) * HL]
                 .reshape(B, HL, L // NBLK, NBLK, D).transpose(0, 3, 1, 2, 4))
        v_pad = np.ones((B, NBLK, HL, L // NBLK, VP), dtype=BF16)
        v_pad[..., :D] = v_blk.astype(BF16)
        v_pad = np.ascontiguousarray(v_pad)
        w_loc = np.ascontiguousarray(np.concatenate(
            [w_qkv[:, lo:hi], w_qkv[:, DIM + lo:DIM + hi],
             w_qkv[:, 2 * DIM + lo:2 * DIM + hi]], axis=1).astype(BF16))
        in_maps.append({
            "x_t": x_t,
            "k_t": k_t,
            "v_pad": v_pad,
            "w_loc": w_loc,
            "b_qk": np.ascontiguousarray(np.stack(
                [b_qkv[lo:hi], b_qkv[DIM + lo:DIM + hi]], axis=1)),
            "b_v": np.ascontiguousarray(b_qkv[2 * DIM + lo:2 * DIM + hi]),
            "wp_loc": np.ascontiguousarray(
                w_proj[lo:hi].reshape(HL, D, DIM).transpose(1, 0, 2)
                .astype(BF16)),
        })
    return in_maps


_NC_CACHE = {}


def get_nc(repeat: int = 1, ablate: str = ""):
    key = (repeat, MMD, ablate)
    if key not in _NC_CACHE:
        _NC_CACHE[key] = build(repeat, ablate)
    return _NC_CACHE[key]


def kernel(x, past_k, past_v, w_qkv, b_qkv, w_proj, b_proj):
    in_maps = shard_inputs(x, past_k, past_v, w_qkv, b_qkv, w_proj)
    nc = get_nc(1)
    try:
        res = run_bass_kernel_spmd(nc, in_maps, core_ids=list(range(N_CORES)))
    except Exception:
        # transient NRT_EXEC_UNIT_UNRECOVERABLE has been observed once on
        # this setup; a plain retry recovers it
        res = run_bass_kernel_spmd(nc, in_maps, core_ids=list(range(N_CORES)))
    out = np.zeros((T, DIM), np.float64)
    for c in range(N_CORES):
        out += np.asarray(res.results[c]["out"]).astype(np.float32)
    out += np.asarray(b_proj, np.float32)
    return out.reshape(B, N, DIM).astype(np.float32)



# revision 13
# speedup vs baseline: 1.1078x; 1.1078x over previous
"""Multi-head attention with KV-cache append, sharded over 8 trn2 NeuronCores.

Reference computation (fp32):
    qkv = x @ w_qkv + b_qkv                  # x [2,256,1024]
    q,k_new,v_new -> [B,H,N,D] (H=16, D=64)
    k_all = cat(past_k, k_new); v_all = cat(past_v, v_new)   # [B,H,8448,64]
    out = softmax(q k^T / sqrt(D)) @ v_all   # non-causal
    return out.merge_heads @ w_proj + b_proj

Sharding: tensor-parallel over heads. Core c owns heads [2c, 2c+1]:
  - past_k/past_v sharded by head, bf16 in HBM (fp8 was tried and costs
    ~3.9% rel err: the attention output is itself an incoherent average,
    so K/V quantization noise does NOT average down relative to it).
    past_k pre-transposed to [B,2,D,L]; past_v re-blocked to
    [B,128,2,L/128,D+1] (partition-major) with the softmax-denominator
    ones column baked in, so every DMA is one fully contiguous
    per-partition run.
  - w_qkv column-split / w_proj row-split per head (bf16); each core
    computes a full-shape bf16 partial of the output projection; host sums
    partials in f64 + b_proj.

Device kernel per core (all engines via the Tile framework):
  q^T/k_new^T/v_new projections on PE, then flash-style streaming over the
  KV cache. The critical loop is latency-pipelined: one [128, 1024] PSUM
  scores tile per superblock (both heads side by side) so the 3-slot PSUM
  ring holds 3 superblocks in flight, exp alternates whole tiles between
  ACT (table exp) and DVE (Schraudolph bit-trick, one tensor_scalar), and
  attn@v consumption trails scores emission by PEND superblocks -- the
  ~1.2us exp latency hides under ~2us of PE work. attn@v accumulates in
  PSUM with the ones column producing the softmax denominator row.
  Epilogue: per-head output projection of the unnormalized attention
  output, per-token denominator scaling (softmax normalization commutes
  with the projection within a head) split ACT/DVE, cross-head sum on
  gpsimd, bf16 output partials.
"""

import sys
for _p in ("/opt/trn_rl_repo", "/root/.axon_site/_ro/trn_rl_repo"):
    if _p not in sys.path:
        sys.path.append(_p)

from contextlib import ExitStack

import numpy as np

import concourse.bass as bass
import concourse.tile as tile
from concourse import bacc, mybir
from concourse.bass_utils import run_bass_kernel_spmd
from concourse.masks import make_identity

N_CORES = 8
B, N, DIM = 2, 256, 1024
H, D = 16, 64
L = 8192
HL = H // N_CORES          # 2 heads per core
DL = HL * D                # 128 local head dims
SCALE = D ** -0.5
T = B * N                  # 512 tokens
CC = DIM // 128            # 8 contraction chunks of the model dim
F32 = mybir.dt.float32
VP = D + 2                 # v padded with per-head ones columns: h0's
                           # softmax-denominator ones at col D, h1's at D+1,
                           # so both heads' denominators land on adjacent
                           # PSUM partitions (64/65) and one PE transpose
                           # per token-chunk recovers both
NBLK = 128                 # kv-cache block (l) per scores matmul
PSB = 2                    # l-blocks per PSUM superblock (per head)
NSB = L // (PSB * NBLK)    # superblocks per batch (cache part)
PEND = 4                   # attn@v trails scores by this many superblocks
KCH = 4                    # kt DMA chunks per batch (earlier stream start)
VCH = 2                    # vt DMA chunks per batch

MMD = mybir.dt.bfloat16
U16 = mybir.dt.uint16
F8 = mybir.dt.float8e4

# Softmax exp is ~56us of ACT work if done entirely on the scalar engine --
# the single largest serial engine cost. Offload a fraction of the exp tiles
# to the DVE using the Schraudolph bit trick in bf16:
#   bf16_bits(exp(s*SCALE)) ~= round(s * EXP_A + EXP_B)
# (one tensor_scalar mult+add, f32 PSUM in -> uint16 out, bitcast to bf16).
# Max rel err ~3.3%, rms ~1.8%, pseudo-random across keys -> averages out in
# the softmax weighted mean (common-mode cancels in the ratio entirely).
EXP_A = float(SCALE * np.log2(np.e) * 128.0)
EXP_B = 16250.5
# of every 16 exp tiles, these slot indices run on DVE (rest on ACT)
DVE_SLOTS = frozenset((1, 3, 5, 7, 9, 11, 13))


def _emit(ctx: ExitStack, tc: tile.TileContext, aps: dict):
    nc = tc.nc
    t_x, t_k, t_v = aps["x_t"], aps["k_t"], aps["v_pad"]
    t_w, t_bq, t_bv = aps["w_loc"], aps["b_q"], aps["b_v"]
    t_wp, t_out = aps["wp_loc"], aps["out"]
    ablate = aps.get("ablate", "")

    singles = ctx.enter_context(tc.tile_pool(name="singles", bufs=1))
    kpool = ctx.enter_context(tc.tile_pool(name="kpool", bufs=2))
    vpool = ctx.enter_context(tc.tile_pool(name="vpool", bufs=2))
    apool = ctx.enter_context(tc.tile_pool(name="apool", bufs=2 * (PEND + 1)))
    epool = ctx.enter_context(tc.tile_pool(name="epool", bufs=4))
    opool = ctx.enter_context(tc.tile_pool(name="opool", bufs=6))
    rpool = ctx.enter_context(tc.tile_pool(name="rpool", bufs=8))
    scp = ctx.enter_context(tc.tile_pool(name="scp", bufs=3, space="PSUM"))
    acp = ctx.enter_context(tc.tile_pool(name="acp", bufs=2, space="PSUM"))

    Exp = mybir.ActivationFunctionType.Exp
    Ident = mybir.ActivationFunctionType.Identity
    Copy = mybir.ActivationFunctionType.Copy

    def body():
        # ---- load x^T, weights, biases ----
        # double-buffered (bufs=2) so the next For_i iteration's loads can
        # land while this iteration still reads the previous buffers
        x_sb = singles.tile([128, CC, T], MMD, tag="x", bufs=2)
        xr = t_x.rearrange("(cc p) t -> p cc t", p=128)
        nc.sync.dma_start(out=x_sb[:, 0:CC // 2], in_=xr[:, 0:CC // 2])
        nc.sync.dma_start(out=x_sb[:, CC // 2:CC], in_=xr[:, CC // 2:CC])
        # w layout [128, 3, CC, DL]: q cols land first so the q projection
        # (the only qkv output the scores stream waits on) starts earliest
        w_sb = singles.tile([128, 3, CC, DL], MMD, tag="w", bufs=2)
        nc.scalar.dma_start(out=w_sb[:, 0], in_=t_w[:, 0])
        bqk_sb = singles.tile([DL, 2], F32, tag="bqk")
        nc.scalar.dma_start(out=bqk_sb, in_=t_bq)
        bq_sb, bk_sb = bqk_sb[:, 0:1], bqk_sb[:, 1:2]
        nc.scalar.dma_start(out=w_sb[:, 1], in_=t_w[:, 1])
        nc.scalar.dma_start(out=w_sb[:, 2], in_=t_w[:, 2])
        # bias-v ahead of the big vt streams on the gpsimd (SWDGE) ring
        bv_sb = singles.tile([128, DL], F32, tag="bv")
        nc.gpsimd.dma_start(
            out=bv_sb,
            in_=bass.AP(tensor=t_bv.tensor, offset=0, ap=[[0, 128], [1, DL]]),
        )
        # prefetch the KV stream in chunks: the scores/attn@v consumers only
        # depend on the chunk covering their l-range, so the b0 stream can
        # start as soon as the first chunk lands instead of the whole 2MB
        kts, vts_all = {}, {}
        for b in range(B):
            kt = kpool.tile([128, L], MMD, tag="k", name=f"kt{b}")
            ktr = t_k[b].rearrange("h d l -> (h d) l")
            for c in range(KCH):
                sl = slice(c * (L // KCH), (c + 1) * (L // KCH))
                nc.sync.dma_start(out=kt[:, sl], in_=ktr[:, sl])
            kts[b] = kt
            vt = vpool.tile([128, HL, L // NBLK, VP], MMD, tag="v",
                            name=f"vt{b}")
            for c in range(VCH):
                sl = slice(c * (L // NBLK // VCH), (c + 1) * (L // NBLK // VCH))
                nc.gpsimd.dma_start(out=vt[:, :, sl], in_=t_v[b][:, :, sl])
            vts_all[b] = vt
        wp_sb = singles.tile([D, HL, DIM], MMD, tag="wp", bufs=2)
        nc.scalar.dma_start(out=wp_sb, in_=t_wp)
        ident = singles.tile([128, 128], F32, tag="ident")
        make_identity(nc, ident)
        ones_sb = singles.tile([128, 1], F32, tag="ones")
        nc.vector.memset(ones_sb, 1.0)
        # [1, 0, 1] pattern: slice [0:2] = h0's ones-columns ([1, 0]),
        # slice [1:3] = h1's ([0, 1])
        oc_sb = singles.tile([128, 3], F32, tag="oc")
        nc.vector.memset(oc_sb[:, 0:1], 1.0)
        nc.vector.memset(oc_sb[:, 1:2], 0.0)
        nc.vector.memset(oc_sb[:, 2:3], 1.0)
        # touch Exp once while ACT is otherwise idle: pulls the ~2.7us
        # ACT_TABLE_LOAD into the startup window
        warm = singles.tile([128, 1], F32, tag="warm")
        nc.scalar.activation(warm, ones_sb, Exp)

        if ablate == "dmaonly":
            for g in range(4):
                for ech in range(2):
                    so = opool.tile([128, 512], MMD, tag="o")
                    nc.vector.tensor_copy(so, x_sb[:, g, :])
                    nc.sync.dma_start(
                        out=t_out[g * 128:(g + 1) * 128,
                                  ech * 512:(ech + 1) * 512],
                        in_=so)
            return

        noexp = ablate in ("noact", "noactpev", "noattn")
        nopev = ablate in ("nopev", "noactpev", "noattn")
        nosc = ablate in ("noattn",)
        at0 = None
        if noexp or ablate == "nopev":
            at0 = singles.tile([128, 2 * PSB * N], MMD, tag="at0")
            nc.vector.memset(at0, 0.001)
        exp_cnt = [0]

        def exp_tile(ps, cols):
            """exp of a whole-superblock scores tile (both heads)."""
            if noexp:
                return at0
            slot = exp_cnt[0] % 16
            exp_cnt[0] += 1
            if slot in DVE_SLOTS:
                atu = apool.tile([128, 2 * PSB * N], U16, tag="a", name="at")
                nc.vector.tensor_scalar(
                    atu[:, 0:cols], ps[:, 0:cols], EXP_A, EXP_B,
                    mybir.AluOpType.mult, mybir.AluOpType.add)
                return atu.bitcast(MMD)
            at = apool.tile([128, 2 * PSB * N], MMD, tag="a", name="at")
            nc.scalar.activation(at[:, 0:cols], ps[:, 0:cols], Exp,
                                 scale=SCALE)
            return at

        # ---- qkv projections ----
        # q^T, k_new^T in [d_local, token] layout (d on partitions)
        psq = scp.tile([128, T], F32, tag="sc")
        for cc in range(CC):
            nc.tensor.matmul(psq, w_sb[:, 0, cc], x_sb[:, cc, :],
                             start=(cc == 0), stop=(cc == CC - 1))
        q_sb = singles.tile([DL, T], MMD, tag="q", bufs=2)
        nc.scalar.activation(q_sb, psq, Ident, bias=bq_sb)

        psk = scp.tile([128, T], F32, tag="sc")
        for cc in range(CC):
            nc.tensor.matmul(psk, w_sb[:, 1, cc], x_sb[:, cc, :],
                             start=(cc == 0), stop=(cc == CC - 1))
        k_sb = singles.tile([DL, T], MMD, tag="kn", bufs=2)
        nc.scalar.activation(k_sb, psk, Ident, bias=bk_sb)

        # v_new in [token, d_local] layout, padded with the ones columns
        vn_sb = {}
        for nch in range(T // 128):  # 4 chunks of 128 tokens
            psv = acp.tile([128, DL], F32, tag="ac")
            for cc in range(CC):
                nc.tensor.matmul(psv, x_sb[:, cc, nch * 128:(nch + 1) * 128],
                                 w_sb[:, 2, cc],
                                 start=(cc == 0), stop=(cc == CC - 1))
            for h in range(HL):
                vn = singles.tile([128, VP], MMD, tag=f"vn{nch}_{h}")
                nc.vector.tensor_add(vn[:, 0:D], psv[:, h * D:(h + 1) * D],
                                     bv_sb[:, h * D:(h + 1) * D])
                nc.vector.tensor_copy(vn[:, D:VP], oc_sb[:, h:h + 2])
                vn_sb[(nch, h)] = vn

        # ---- streaming attention over the KV cache ----
        for b in range(B):
            kt, vt = kts[b], vts_all[b]
            # acc [VP, 2*N]: halves accumulate the even (l in [0,64)) and
            # odd (l in [64,128)) partition-halves of each kv block via two
            # K=64 matmuls on disjoint PE row groups -- they run
            # concurrently (measured 3.07x for 4-way row tiling); the
            # epilogue sums the halves on DVE
            acc = {}
            for h in range(HL):
                acc[(b, h)] = acp.tile([VP, 2 * N], F32, tag="ac",
                                       name=f"acc{b}{h}")

            # software pipeline: attn@v of superblock i is emitted after the
            # scores of superblock i+PEND, so the exp latency (~1.2us) hides
            # under several superblocks of PE work
            pend = []

            def emit_attnv(entry):
                at, blocks = entry
                for (h, lhs_rhs) in blocks:
                    vt_ap, at_cols, start, stop = lhs_rhs
                    nc.tensor.matmul(
                        acc[(b, h)][:, 0:N], vt_ap,
                        at[:, at_cols[0]:at_cols[1]],
                        start=start, stop=stop, skip_group_check=True)

            def flush(limit):
                while len(pend) > limit:
                    emit_attnv(pend.pop(0))

            for psb in range(NSB):
                # one [128, 2*PSB*N] PSUM tile per superblock: h0 in the
                # low half, h1 in the high half -> the 3-slot PSUM ring
                # keeps 3 superblocks in flight
                ps = scp.tile([128, 2 * PSB * N], F32, tag="sc", name="ps")
                if not nosc:
                    for j in range(PSB):
                        lo = (psb * PSB + j) * NBLK
                        for h in range(HL):
                            # the two heads' scores matmuls sit on disjoint
                            # PE row groups (partitions 0-63 / 64-127) and
                            # run concurrently
                            nc.tensor.matmul(
                                ps[:, (h * PSB + j) * N:(h * PSB + j + 1) * N],
                                kt[h * D:(h + 1) * D, lo:lo + NBLK],
                                q_sb[h * D:(h + 1) * D, b * N:(b + 1) * N],
                                start=True, stop=True)
                at = exp_tile(ps, 2 * PSB * N)
                if not nopev:
                    blocks = []
                    for j in range(PSB):
                        jj = psb * PSB + j
                        for h in range(HL):
                            blocks.append((h, (
                                vt[:, h, jj, :],
                                ((h * PSB + j) * N, (h * PSB + j + 1) * N),
                                jj == 0, False)))
                    pend.append((at, blocks))
                flush(PEND)
            # new tokens (the appended k_new/v_new of this batch)
            ps = scp.tile([128, 2 * PSB * N], F32, tag="sc", name="ps")
            if not nosc:
                for j in range(2):
                    for h in range(HL):
                        nc.tensor.matmul(
                            ps[:, (h * 2 + j) * N:(h * 2 + j + 1) * N],
                            k_sb[h * D:(h + 1) * D,
                                 b * N + j * NBLK:b * N + (j + 1) * NBLK],
                            q_sb[h * D:(h + 1) * D, b * N:(b + 1) * N],
                            start=True, stop=True)
            at = exp_tile(ps, 4 * N)
            blocks = []
            for j in range(2):
                for h in range(HL):
                    blocks.append((h, (
                        vn_sb[(b * 2 + j, h)],
                        ((h * 2 + j) * N, (h * 2 + j + 1) * N),
                        nopev and j == 0, j == 1)))
            pend.append((at, blocks))
            flush(0)

            # ---- per-batch epilogue ----
            # h0's denominator sits on acc partition 64, h1's on 65 (the
            # per-head ones columns), so one [2, 128] PE transpose per
            # token-chunk recovers both heads' denominators at once
            uns = {}
            den = epool.tile([VP, N], F32, tag="den")
            for h in range(HL):
                un = epool.tile([D, N], MMD, tag="un")
                nc.vector.tensor_copy(un, acc[(b, h)][0:D, 0:N])
                uns[h] = un
            # engine PSUM reads need a 32-aligned base partition: copy h1's
            # rows [64:66] (row 64 is h1's zero column), then overwrite row
            # 64 with h0's denominator
            nc.vector.tensor_copy(den[D:VP, :], acc[(b, 1)][D:VP, 0:N])
            nc.vector.tensor_copy(den[D:D + 1, :], acc[(b, 0)][D:D + 1, 0:N])
            recips = {}
            for tch in range(N // 128):
                tp = scp.tile([128, 2], F32, tag="sc")
                nc.tensor.transpose(
                    tp, den[D:VP, tch * 128:(tch + 1) * 128],
                    ident[D:VP, D:VP])
                rc = rpool.tile([128, 2], F32, tag="rc")
                nc.vector.reciprocal(rc, tp)
                recips[tch] = rc

            for tch in range(N // 128):
                g = b * (N // 128) + tch
                og = opool.tile([128, DIM], MMD, tag="og")
                for ech in range(DIM // 512):
                    pps = []
                    for h in range(HL):
                        pp = scp.tile([128, 512], F32, tag="sc")
                        nc.tensor.matmul(
                            pp,
                            uns[h][:, tch * 128:(tch + 1) * 128],
                            wp_sb[:, h, ech * 512:(ech + 1) * 512],
                            start=True, stop=True)
                        pps.append(pp)
                    # per-token denominator scaling, one head on ACT (Copy
                    # with a per-partition scale AP) and one on DVE
                    s0 = opool.tile([128, 512], F32, tag="o")
                    nc.scalar.activation(s0, pps[0], Copy,
                                         scale=recips[tch][:, 0:1])
                    s1 = opool.tile([128, 512], F32, tag="o")
                    nc.vector.tensor_scalar_mul(s1, pps[1],
                                                recips[tch][:, 1:2])
                    # cross-head sum on gpsimd (SBUF-only engine)
                    nc.gpsimd.tensor_add(og[:, ech * 512:(ech + 1) * 512],
                                         s0, s1)
                nc.sync.dma_start(
                    out=t_out[g * 128:(g + 1) * 128, :], in_=og)

    repeat = aps["repeat"]
    if repeat > 1:
        with tc.For_i(0, repeat, 1):
            body()
    else:
        body()


def build(repeat: int = 1, ablate: str = ""):
    """Build + bass-compile the SPMD program (one NeuronCore's view)."""
    nc = bacc.Bacc("TRN2", target_bir_lowering=False, debug=False,
                   num_devices=N_CORES)
    aps = {
        "x_t": nc.dram_tensor("x_t", [DIM, T], MMD, kind="ExternalInput").ap(),
        "k_t": nc.dram_tensor("k_t", [B, HL, D, L], MMD, kind="ExternalInput").ap(),
        "v_pad": nc.dram_tensor("v_pad", [B, 128, HL, L // NBLK, VP], MMD, kind="ExternalInput").ap(),
        "w_loc": nc.dram_tensor("w_loc", [128, 3, CC, DL], MMD, kind="ExternalInput").ap(),
        "b_q": nc.dram_tensor("b_qk", [DL, 2], F32, kind="ExternalInput").ap(),
        "b_v": nc.dram_tensor("b_v", [DL], F32, kind="ExternalInput").ap(),
        "wp_loc": nc.dram_tensor("wp_loc", [D, HL, DIM], MMD, kind="ExternalInput").ap(),
        "out": nc.dram_tensor("out", [T, DIM], MMD, kind="ExternalOutput").ap(),
        "repeat": repeat,
        "ablate": ablate,
    }
    with tile.TileContext(nc) as tc:
        with ExitStack() as ctx:
            _emit(ctx, tc, aps)
    nc.compile()
    return nc


def shard_inputs(x, past_k, past_v, w_qkv, b_qkv, w_proj):
    """Full inputs -> list of 8 per-core input maps (head-sharded)."""
    import ml_dtypes
    BF16 = ml_dtypes.bfloat16

    x = np.asarray(x, np.float32)
    past_k = np.asarray(past_k, np.float32)
    past_v = np.asarray(past_v, np.float32)
    w_qkv = np.asarray(w_qkv, np.float32)
    b_qkv = np.asarray(b_qkv, np.float32)
    w_proj = np.asarray(w_proj, np.float32)

    x_t = np.ascontiguousarray(x.reshape(T, DIM).T.astype(BF16))
    in_maps = []
    for c in range(N_CORES):
        lo, hi = c * DL, (c + 1) * DL
        k_t = np.ascontiguousarray(
            past_k[:, c * HL:(c + 1) * HL].transpose(0, 1, 3, 2).astype(BF16))
        # [B, HL, L, D] -> [B, HL, 128, L//128, D+2] with per-head ones
        # columns (softmax-denominator trick): h0's ones at d=D, h1's at
        # d=D+1 so both denominators land on adjacent PSUM partitions.
        # layout [B, p=128, HL, L//128, D+2]: partition-major so the SBUF
        # destination AP matches the source element order exactly
        v_blk = (past_v[:, c * HL:(c + 1) * HL]
                 .reshape(B, HL, L // NBLK, NBLK, D).transpose(0, 3, 1, 2, 4))
        v_pad = np.zeros((B, NBLK, HL, L // NBLK, VP), dtype=BF16)
        v_pad[..., :D] = v_blk.astype(BF16)
        for h in range(HL):
            v_pad[:, :, h, :, D + h] = 1
        v_pad = np.ascontiguousarray(v_pad)
        # w layout [p=128, 3, CC, DL]: w_loc[p, g, cc, d] =
        # w_qkv[cc*128 + p, g*DIM + lo + d]
        w_loc = np.ascontiguousarray(
            np.stack([w_qkv[:, lo:hi], w_qkv[:, DIM + lo:DIM + hi],
                      w_qkv[:, 2 * DIM + lo:2 * DIM + hi]], axis=1)
            .reshape(CC, 128, 3, DL).transpose(1, 2, 0, 3).astype(BF16))
        in_maps.append({
            "x_t": x_t,
            "k_t": k_t,
            "v_pad": v_pad,
            "w_loc": w_loc,
            "b_qk": np.ascontiguousarray(np.stack(
                [b_qkv[lo:hi], b_qkv[DIM + lo:DIM + hi]], axis=1)),
            "b_v": np.ascontiguousarray(b_qkv[2 * DIM + lo:2 * DIM + hi]),
            "wp_loc": np.ascontiguousarray(
                w_proj[lo:hi].reshape(HL, D, DIM).transpose(1, 0, 2)
                .astype(BF16)),
        })
    return in_maps


_NC_CACHE = {}


def get_nc(repeat: int = 1, ablate: str = ""):
    key = (repeat, MMD, ablate)
    if key not in _NC_CACHE:
        _NC_CACHE[key] = build(repeat, ablate)
    return _NC_CACHE[key]


def kernel(x, past_k, past_v, w_qkv, b_qkv, w_proj, b_proj):
    in_maps = shard_inputs(x, past_k, past_v, w_qkv, b_qkv, w_proj)
    nc = get_nc(1)
    try:
        res = run_bass_kernel_spmd(nc, in_maps, core_ids=list(range(N_CORES)))
    except Exception:
        # transient NRT_EXEC_UNIT_UNRECOVERABLE has been observed once on
        # this setup; a plain retry recovers it
        res = run_bass_kernel_spmd(nc, in_maps, core_ids=list(range(N_CORES)))
    out = np.zeros((T, DIM), np.float64)
    for c in range(N_CORES):
        out += np.asarray(res.results[c]["out"]).astype(np.float32)
    out += np.asarray(b_proj, np.float32)
    return out.reshape(B, N, DIM).astype(np.float32)

